# revision 31
# baseline (speedup 1.0000x reference)
"""FP8Linear Trainium2 kernel.

Computes out = quant_e4m3(x) @ quant_e4m3(w).T in fp32, distributed over 8
NeuronCores as a 2x4 grid (x rows x w rows). Per core:

  xT_in [2048, 4096] bf16, wT_in [2048, 2048] bf16 -> out [4096, 2048] f32

Host-side staging (layout + lossless re-encoding, exact):
  - both operands transposed so the contraction dim c is the on-chip
    partition dim (no transposes run on the device at all);
  - f32 -> bf16 with ROUND-TO-ODD (truncate + sticky bit into the lsb).
    RNE(bf16_RO(v) -> e4m3) == RNE(v -> e4m3) exactly (double-rounding is
    exact when the intermediate format has >= 2 more mantissa bits than
    the target: bf16 has 8, e4m3 needs 3+2), so the device's ACT
    bf16->fp8 quantize reproduces the reference f32->fp8 bit-for-bit
    while input DMA traffic halves.

Device pipeline:
  loads: bf16 c-chunk slabs -> ACT quantize bf16->fp8e4 (the rounding
     step) into resident fp8 tiles XT [128c, 16s, 4096m],
     WT [128c, 16s, 2048o]
  matmul: fp8 DoubleRow (c-chunk pairs on partitions), fp32 PSUM, N=512
     tiles, 8 PSUM banks, ACT/DVE drains, 512 KiB f32 output DMAs.

Schedule: w's first o-half + x slab group 0 load first (6 MiB) and the
first matmul group chases them c-ascending; w's second o-half trickles
in behind x group 2; the o-half-1 matmul groups are emitted late, when
everything they need is already resident. Loads ride the two HWDGE
FIFOs (sync + scalar), stores ride the SWDGE queue — no FIFO ever mixes
loads and stores, so neither can head-of-line block the other.
"""

import numpy as np
import ml_dtypes

# ---- problem constants (hardcoded per task contract) ----
A_DIM, B_DIM, C_DIM, OUT_DIM = 4, 2048, 2048, 8192
M_FULL = A_DIM * B_DIM  # 8192
GRID_M, GRID_O = 2, 4
N_CORES = GRID_M * GRID_O
M_CORE = M_FULL // GRID_M   # 4096
O_CORE = OUT_DIM // GRID_O  # 2048

P = 128


def build_nc(m_core=M_CORE, o_core=O_CORE, c_dim=C_DIM,
             m_slab=512, n_tile=512, mm_psum_bufs=8):
    """Build the single-core Bass program (same program runs SPMD on 8 cores)."""
    import contextlib

    import concourse.bacc as bacc
    import concourse.mybir as mybir
    import concourse.tile as tile

    bf16 = mybir.dt.bfloat16
    f32 = mybir.dt.float32
    fp8 = mybir.dt.float8e4
    Copy = mybir.ActivationFunctionType.Copy
    DR = mybir.MatmulPerfMode.DoubleRow

    S = c_dim // P              # c-chunks (16)
    SP = S // 2                 # DoubleRow pairs (8)
    MG = m_core // m_slab       # x slab groups (8)
    MWG = m_slab // P           # m windows per slab group (4)
    NT = o_core // n_tile       # o tiles (4)
    NTH = NT // 2               # o tiles per half (2)
    o_half = o_core // 2        # 1024

    nc = bacc.Bacc(None, target_bir_lowering=False, debug=False)
    x_in = nc.declare_dram_parameter("x_in", [c_dim, m_core], bf16, isOutput=False)
    w_in = nc.declare_dram_parameter("w_in", [c_dim, o_core], bf16, isOutput=False)
    out = nc.declare_dram_parameter("out", [m_core, o_core], f32, isOutput=True)

    with tile.TileContext(nc) as tc:
        with contextlib.ExitStack() as ctx:
            wstg = ctx.enter_context(tc.tile_pool(name="wstg", bufs=16))
            xstg = ctx.enter_context(tc.tile_pool(name="xstg", bufs=24))
            xres = ctx.enter_context(tc.tile_pool(name="xres", bufs=1))
            wres = ctx.enter_context(tc.tile_pool(name="wres", bufs=1))
            mmp = ctx.enter_context(
                tc.tile_pool(name="mmp", bufs=mm_psum_bufs, space="PSUM"))
            osb = ctx.enter_context(tc.tile_pool(name="osb", bufs=6))

            # resident fp8 operands, c on partitions
            XT = xres.tile([P, S, m_core], fp8)
            WT = wres.tile([P, S, o_core], fp8)

            def w_slab(s, o0, width, q, qeng):
                """w.T block [s*128c, o0..o0+width): load bf16, quantize into
                WT. The prefix loads o-quarters so the first matmul pass
                needs only 2 MiB of w."""
                wst = wstg.tile([P, o_half], bf16, tag="wst", name="wst")
                src = w_in[s * P:(s + 1) * P, o0:o0 + width]
                if q == 0:
                    nc.sync.dma_start(out=wst[:, 0:width], in_=src)
                else:
                    nc.scalar.dma_start(out=wst[:, 0:width], in_=src)
                dst = WT[:, s, o0:o0 + width]
                if qeng == 0:
                    nc.scalar.activation(dst, wst[:, 0:width], Copy)
                elif qeng == 1:
                    nc.vector.tensor_copy(out=dst, in_=wst[:, 0:width])

            def x_slab(s, mg, q, qeng=0):
                """x.T block [s*128c, mg*m_slab m): load bf16, quantize into XT.

                Engine split: in the prefix (before any drains exist) quants
                alternate ACT/DVE; in steady state they stay on ACT so the
                DVE FIFO carries ONLY psum drains and nothing can delay a
                drain (a blocked drain holds a PSUM bank and stalls PE)."""
                m0 = mg * m_slab
                xst = xstg.tile([P, m_slab], bf16, tag="xst", name="xst")
                src = x_in[s * P:(s + 1) * P, m0:m0 + m_slab]
                if q == 0:
                    nc.sync.dma_start(out=xst[:], in_=src)
                else:
                    nc.scalar.dma_start(out=xst[:], in_=src)
                dst = XT[:, s, m0:m0 + m_slab]
                if qeng == 0:
                    nc.scalar.activation(dst, xst[:], Copy)
                else:
                    nc.vector.tensor_copy(out=dst, in_=xst[:])

            def mm_half(mw, oh, tail=False):
                """One 128-row m window x one 1024-col o half: 8 sp x 2 nt
                DoubleRow matmuls, drain 2 psum tiles, 512 KiB store.

                Steady state keeps all drains on DVE (so nothing can delay a
                drain and hold a PSUM bank) and all stores on SWDGE. In the
                tail, ACT and the scalar HWDGE queue are idle, so drains and
                stores spread across both to shorten the endgame."""
                ps = [mmp.tile([P, n_tile], f32, tag="mm_psum", name="mm_psum")
                      for _ in range(NTH)]
                for sp in range(SP):
                    lhsT = XT[:, 2 * sp:2 * sp + 2, mw * P:(mw + 1) * P]
                    for j in range(NTH):
                        nt = NTH * oh + j
                        nc.tensor.matmul(
                            ps[j][:], lhsT,
                            WT[:, 2 * sp:2 * sp + 2, nt * n_tile:(nt + 1) * n_tile],
                            start=(sp == 0), stop=(sp == SP - 1),
                            perf_mode=DR)
                ot = osb.tile([P, o_half], f32, tag="ot", name="ot")
                for j in range(NTH):
                    dst = ot[:, j * n_tile:(j + 1) * n_tile]
                    if tail and j % 2 == 0:
                        nc.scalar.activation(dst, ps[j][:], Copy)
                    else:
                        nc.vector.tensor_copy(out=dst, in_=ps[j][:])
                odst = out[mw * P:(mw + 1) * P, oh * o_half:(oh + 1) * o_half]
                if tail and mw % 2 == 0:
                    nc.scalar.dma_start(out=odst, in_=ot[:])
                else:
                    nc.gpsimd.dma_start(out=odst, in_=ot[:])

            def mm_nt(mw, nt, ot, store):
                """Single o-tile (512-col) psum group for one m window: the
                mg0 prefix runs as two of these per window so the first pass
                only needs a 2 MiB o-quarter of w. Both passes drain into the
                same shared ot tile; the second pass stores it."""
                ps = mmp.tile([P, n_tile], f32, tag="mm_psum", name="mm_psum")
                for sp in range(SP):
                    lhsT = XT[:, 2 * sp:2 * sp + 2, mw * P:(mw + 1) * P]
                    nc.tensor.matmul(
                        ps[:], lhsT,
                        WT[:, 2 * sp:2 * sp + 2, nt * n_tile:(nt + 1) * n_tile],
                        start=(sp == 0), stop=(sp == SP - 1),
                        perf_mode=DR)
                nc.vector.tensor_copy(
                    out=ot[:, (nt % NTH) * n_tile:(nt % NTH + 1) * n_tile],
                    in_=ps[:])
                if store:
                    oh = nt // NTH
                    nc.gpsimd.dma_start(
                        out=out[mw * P:(mw + 1) * P,
                                oh * o_half:(oh + 1) * o_half],
                        in_=ot[:])

            def mg_windows(mg):
                return range(mg * MWG, (mg + 1) * MWG)

            # ---- prefix: first o-half of w + x groups 0-1, c-ascending on both
            # HWDGE queues; the first matmul group chases them. Loads for wave
            # mg+1 are always emitted BEFORE group mg's matmuls so each wave's
            # quantizes drain during the previous group's matmul stream. w's
            # second o-half trickles behind x groups 2-3; o-half-1 matmul
            # groups backfill once everything they need is resident ----
            # wave 0: first o-QUARTER of w + x group 0 (4 MiB) — the minimum
            # that lets single-o-tile matmul groups saturate PE
            for s in range(S):
                w_slab(s, 0, n_tile, q=s % 2, qeng=s % 2)
                x_slab(s, 0, q=(s + 1) % 2, qeng=(s + 1) % 2)
            # wave 1: second o-quarter + x group 1; quants all-ACT so the DVE
            # FIFO holds nothing ahead of the mg0 drains
            for s in range(S):
                w_slab(s, n_tile, n_tile, q=s % 2, qeng=0)
                x_slab(s, 1, q=(s + 1) % 2, qeng=0)
            # mg0 as two o-quarter passes sharing one output tile per window
            ots0 = {}
            for mw in mg_windows(0):
                ots0[mw] = osb.tile([P, o_half], f32, tag="ot", name="ot")
                mm_nt(mw, 0, ots0[mw], store=False)
            for s in range(S):
                x_slab(s, 2, q=s % 2)
            for mw in mg_windows(0):
                mm_nt(mw, 1, ots0[mw], store=True)
            for mw in mg_windows(1):
                mm_half(mw, 0)

            # steady state: prefetch wave mg+1 before group mg; w's second
            # o-half trickles at the head of the mg3/mg4 waves (quants on
            # DVE, ahead of that wave's drains but behind nothing else)
            for mg in range(2, MG):
                if mg + 1 < MG:
                    if mg in (3, 4):
                        for s in range(S):
                            if (s + mg) % 2 == 0:
                                w_slab(s, o_half, o_half,
                                       q=s % 2, qeng=1)
                    for s in range(S):
                        x_slab(s, mg + 1, q=(s + mg) % 2)
                for mw in mg_windows(mg):
                    mm_half(mw, 0)
                if mg >= 4:
                    for mw in mg_windows(mg - 4):
                        mm_half(mw, 1)
            for mg in range(MG - 4, MG):
                for mw in mg_windows(mg):
                    mm_half(mw, 1, tail=(mg >= MG - 2))

    nc.finalize()
    return nc


def _round_to_odd_bf16(a):
    """f32 -> bf16 by truncation with the sticky bit ORed into the lsb.

    RNE(result -> e4m3) == RNE(a -> e4m3) exactly (no double rounding).
    """
    u = np.ascontiguousarray(a, dtype=np.float32).view(np.uint32)
    hi = (u >> 16).astype(np.uint16)
    hi |= ((u & 0xFFFF) != 0).astype(np.uint16)
    return hi.view(ml_dtypes.bfloat16)


_NC = None


def _get_nc():
    global _NC
    if _NC is None:
        _NC = build_nc()
    return _NC


def kernel(input, weight, input_scale_e4m3=None, weight_scale_e4m3=None,
           **_unused):
    from concourse.bass_utils import run_bass_kernel_spmd

    x = np.asarray(input, dtype=np.float32).reshape(M_FULL, C_DIM)
    w = np.asarray(weight, dtype=np.float32)
    s_in = float(np.asarray(input_scale_e4m3)) if input_scale_e4m3 is not None else 1.0
    s_w = float(np.asarray(weight_scale_e4m3)) if weight_scale_e4m3 is not None else 1.0

    # reference semantics: round(x*s)/s etc.; fold scales on host (exact)
    if s_in != 1.0:
        x = x * s_in
    if s_w != 1.0:
        w = w * s_w

    # host-side staging: round-to-odd bf16 (exact w.r.t. the later fp8
    # RNE quantize) + transpose so c is the on-chip contraction dim
    xb = _round_to_odd_bf16(x)
    wb = _round_to_odd_bf16(w)
    xT = [np.ascontiguousarray(xb[mi * M_CORE:(mi + 1) * M_CORE].T)
          for mi in range(GRID_M)]
    wT = [np.ascontiguousarray(wb[oj * O_CORE:(oj + 1) * O_CORE].T)
          for oj in range(GRID_O)]

    nc = _get_nc()
    in_maps = []
    for mi in range(GRID_M):
        for oj in range(GRID_O):
            in_maps.append({"x_in": xT[mi], "w_in": wT[oj]})
    res = run_bass_kernel_spmd(nc, in_maps, core_ids=list(range(N_CORES)))

    out = np.empty((M_FULL, OUT_DIM), np.float32)
    for k, r in enumerate(res.results):
        mi, oj = divmod(k, GRID_O)
        out[mi * M_CORE:(mi + 1) * M_CORE, oj * O_CORE:(oj + 1) * O_CORE] = r["out"]

    inv = 1.0 / (s_in * s_w)
    if inv != 1.0:
        out = out * inv
    return out.reshape(A_DIM, B_DIM, OUT_DIM)


# revision 32
# speedup vs baseline: 1.0080x; 1.0080x over previous
"""FP8Linear Trainium2 kernel.

Computes out = quant_e4m3(x) @ quant_e4m3(w).T in fp32, distributed over 8
NeuronCores as a 2x4 grid (x rows x w rows). Per core:

  xT_in [2048, 4096] bf16, wT_in [2048, 2048] bf16 -> out [4096, 2048] f32

Host-side staging (layout + lossless re-encoding, exact):
  - both operands transposed so the contraction dim c is the on-chip
    partition dim (no transposes run on the device at all);
  - f32 -> bf16 with ROUND-TO-ODD (truncate + sticky bit into the lsb).
    RNE(bf16_RO(v) -> e4m3) == RNE(v -> e4m3) exactly (double-rounding is
    exact when the intermediate format has >= 2 more mantissa bits than
    the target: bf16 has 8, e4m3 needs 3+2), so the device's ACT
    bf16->fp8 quantize reproduces the reference f32->fp8 bit-for-bit
    while input DMA traffic halves.

Device pipeline:
  loads: bf16 c-chunk slabs -> ACT quantize bf16->fp8e4 (the rounding
     step) into resident fp8 tiles XT [128c, 16s, 4096m],
     WT [128c, 16s, 2048o]
  matmul: fp8 DoubleRow (c-chunk pairs on partitions), fp32 PSUM, N=512
     tiles, 8 PSUM banks, ACT/DVE drains, 512 KiB f32 output DMAs.

Schedule: w's first o-half + x slab group 0 load first (6 MiB) and the
first matmul group chases them c-ascending; w's second o-half trickles
in behind x group 2; the o-half-1 matmul groups are emitted late, when
everything they need is already resident. Loads ride the two HWDGE
FIFOs (sync + scalar), stores ride the SWDGE queue — no FIFO ever mixes
loads and stores, so neither can head-of-line block the other.
"""

import numpy as np
import ml_dtypes

# ---- problem constants (hardcoded per task contract) ----
A_DIM, B_DIM, C_DIM, OUT_DIM = 4, 2048, 2048, 8192
M_FULL = A_DIM * B_DIM  # 8192
GRID_M, GRID_O = 2, 4
N_CORES = GRID_M * GRID_O
M_CORE = M_FULL // GRID_M   # 4096
O_CORE = OUT_DIM // GRID_O  # 2048

P = 128


def build_nc(m_core=M_CORE, o_core=O_CORE, c_dim=C_DIM,
             m_slab=512, n_tile=512, mm_psum_bufs=8):
    """Build the single-core Bass program (same program runs SPMD on 8 cores)."""
    import contextlib

    import concourse.bacc as bacc
    import concourse.mybir as mybir
    import concourse.tile as tile

    bf16 = mybir.dt.bfloat16
    f32 = mybir.dt.float32
    fp8 = mybir.dt.float8e4
    Copy = mybir.ActivationFunctionType.Copy
    DR = mybir.MatmulPerfMode.DoubleRow

    S = c_dim // P              # c-chunks (16)
    SP = S // 2                 # DoubleRow pairs (8)
    MG = m_core // m_slab       # x slab groups (8)
    MWG = m_slab // P           # m windows per slab group (4)
    NT = o_core // n_tile       # o tiles (4)
    NTH = NT // 2               # o tiles per half (2)
    o_half = o_core // 2        # 1024

    nc = bacc.Bacc(None, target_bir_lowering=False, debug=False)
    x_in = nc.declare_dram_parameter("x_in", [c_dim, m_core], bf16, isOutput=False)
    w_in = nc.declare_dram_parameter("w_in", [c_dim, o_core], bf16, isOutput=False)
    out = nc.declare_dram_parameter("out", [m_core, o_core], f32, isOutput=True)

    with tile.TileContext(nc) as tc:
        with contextlib.ExitStack() as ctx:
            wstg = ctx.enter_context(tc.tile_pool(name="wstg", bufs=16))
            xstg = ctx.enter_context(tc.tile_pool(name="xstg", bufs=24))
            xres = ctx.enter_context(tc.tile_pool(name="xres", bufs=1))
            wres = ctx.enter_context(tc.tile_pool(name="wres", bufs=1))
            mmp = ctx.enter_context(
                tc.tile_pool(name="mmp", bufs=mm_psum_bufs, space="PSUM"))
            osb = ctx.enter_context(tc.tile_pool(name="osb", bufs=6))

            # resident fp8 operands, c on partitions
            XT = xres.tile([P, S, m_core], fp8)
            WT = wres.tile([P, S, o_core], fp8)

            def w_slab(s, o0, width, q, qeng):
                """w.T block [s*128c, o0..o0+width): load bf16, quantize into
                WT. The prefix loads o-quarters so the first matmul pass
                needs only 2 MiB of w."""
                wst = wstg.tile([P, o_half], bf16, tag="wst", name="wst")
                src = w_in[s * P:(s + 1) * P, o0:o0 + width]
                if q == 0:
                    nc.sync.dma_start(out=wst[:, 0:width], in_=src)
                else:
                    nc.scalar.dma_start(out=wst[:, 0:width], in_=src)
                dst = WT[:, s, o0:o0 + width]
                if qeng == 0:
                    nc.scalar.activation(dst, wst[:, 0:width], Copy)
                elif qeng == 1:
                    nc.vector.tensor_copy(out=dst, in_=wst[:, 0:width])

            def x_slab(s, mg, q, qeng=0):
                """x.T block [s*128c, mg*m_slab m): load bf16, quantize into XT.

                Engine split: in the prefix (before any drains exist) quants
                alternate ACT/DVE; in steady state they stay on ACT so the
                DVE FIFO carries ONLY psum drains and nothing can delay a
                drain (a blocked drain holds a PSUM bank and stalls PE)."""
                m0 = mg * m_slab
                xst = xstg.tile([P, m_slab], bf16, tag="xst", name="xst")
                src = x_in[s * P:(s + 1) * P, m0:m0 + m_slab]
                if q == 0:
                    nc.sync.dma_start(out=xst[:], in_=src)
                else:
                    nc.scalar.dma_start(out=xst[:], in_=src)
                dst = XT[:, s, m0:m0 + m_slab]
                if qeng == 0:
                    nc.scalar.activation(dst, xst[:], Copy)
                else:
                    nc.vector.tensor_copy(out=dst, in_=xst[:])

            def mm_half(mw, oh, tail=False):
                """One 128-row m window x one 1024-col o half: 8 sp x 2 nt
                DoubleRow matmuls, drain 2 psum tiles, 512 KiB store.

                Steady state keeps all drains on DVE (so nothing can delay a
                drain and hold a PSUM bank) and all stores on SWDGE. In the
                tail, ACT and the scalar HWDGE queue are idle, so drains and
                stores spread across both to shorten the endgame."""
                ps = [mmp.tile([P, n_tile], f32, tag="mm_psum", name="mm_psum")
                      for _ in range(NTH)]
                for sp in range(SP):
                    lhsT = XT[:, 2 * sp:2 * sp + 2, mw * P:(mw + 1) * P]
                    for j in range(NTH):
                        nt = NTH * oh + j
                        nc.tensor.matmul(
                            ps[j][:], lhsT,
                            WT[:, 2 * sp:2 * sp + 2, nt * n_tile:(nt + 1) * n_tile],
                            start=(sp == 0), stop=(sp == SP - 1),
                            perf_mode=DR)
                ot = osb.tile([P, o_half], f32, tag="ot", name="ot")
                for j in range(NTH):
                    dst = ot[:, j * n_tile:(j + 1) * n_tile]
                    if tail and j % 2 == 0:
                        nc.scalar.activation(dst, ps[j][:], Copy)
                    else:
                        nc.vector.tensor_copy(out=dst, in_=ps[j][:])
                odst = out[mw * P:(mw + 1) * P, oh * o_half:(oh + 1) * o_half]
                if tail and mw % 2 == 0:
                    nc.scalar.dma_start(out=odst, in_=ot[:])
                else:
                    nc.gpsimd.dma_start(out=odst, in_=ot[:])

            def mm_nt(mw, nt, ot, store):
                """Single o-tile (512-col) psum group for one m window: the
                mg0 prefix runs as two of these per window so the first pass
                only needs a 2 MiB o-quarter of w. Both passes drain into the
                same shared ot tile; the second pass stores it."""
                ps = mmp.tile([P, n_tile], f32, tag="mm_psum", name="mm_psum")
                for sp in range(SP):
                    lhsT = XT[:, 2 * sp:2 * sp + 2, mw * P:(mw + 1) * P]
                    nc.tensor.matmul(
                        ps[:], lhsT,
                        WT[:, 2 * sp:2 * sp + 2, nt * n_tile:(nt + 1) * n_tile],
                        start=(sp == 0), stop=(sp == SP - 1),
                        perf_mode=DR)
                nc.vector.tensor_copy(
                    out=ot[:, (nt % NTH) * n_tile:(nt % NTH + 1) * n_tile],
                    in_=ps[:])
                if store:
                    oh = nt // NTH
                    nc.gpsimd.dma_start(
                        out=out[mw * P:(mw + 1) * P,
                                oh * o_half:(oh + 1) * o_half],
                        in_=ot[:])

            def mg_windows(mg):
                return range(mg * MWG, (mg + 1) * MWG)

            # ---- prefix: first o-half of w + x groups 0-1, c-ascending on both
            # HWDGE queues; the first matmul group chases them. Loads for wave
            # mg+1 are always emitted BEFORE group mg's matmuls so each wave's
            # quantizes drain during the previous group's matmul stream. w's
            # second o-half trickles behind x groups 2-3; o-half-1 matmul
            # groups backfill once everything they need is resident ----
            # wave 0: first o-QUARTER of w + x group 0 (4 MiB) — the minimum
            # that lets single-o-tile matmul groups saturate PE
            for s in range(S):
                w_slab(s, 0, n_tile, q=s % 2, qeng=s % 2)
                x_slab(s, 0, q=(s + 1) % 2, qeng=(s + 1) % 2)
            # wave 1: second o-quarter + x group 1; quants all-ACT so the DVE
            # FIFO holds nothing ahead of the mg0 drains
            for s in range(S):
                w_slab(s, n_tile, n_tile, q=s % 2, qeng=0)
                x_slab(s, 1, q=(s + 1) % 2, qeng=0)
            # mg0 as two o-quarter passes sharing one output tile per window
            ots0 = {}
            for mw in mg_windows(0):
                ots0[mw] = osb.tile([P, o_half], f32, tag="ot", name="ot")
                mm_nt(mw, 0, ots0[mw], store=False)
            for s in range(S):
                x_slab(s, 2, q=s % 2)
            for mw in mg_windows(0):
                mm_nt(mw, 1, ots0[mw], store=True)
            for mw in mg_windows(1):
                mm_half(mw, 0)

            # steady state: prefetch wave mg+1 before group mg; w's second
            # o-half trickles at the head of the mg3/mg4 waves (quants on
            # DVE, ahead of that wave's drains but behind nothing else)
            for mg in range(2, MG):
                if mg + 1 < MG:
                    if mg in (2, 3):
                        for s in range(S):
                            if (s + mg) % 2 == 0:
                                w_slab(s, o_half, o_half,
                                       q=s % 2, qeng=1)
                    for s in range(S):
                        x_slab(s, mg + 1, q=(s + mg) % 2)
                for mw in mg_windows(mg):
                    mm_half(mw, 0)
                if mg >= 4:
                    for mw in mg_windows(mg - 4):
                        mm_half(mw, 1)
            for mg in range(MG - 4, MG):
                for mw in mg_windows(mg):
                    mm_half(mw, 1, tail=(mg >= MG - 2))

    nc.finalize()
    return nc


def _round_to_odd_bf16(a):
    """f32 -> bf16 by truncation with the sticky bit ORed into the lsb.

    RNE(result -> e4m3) == RNE(a -> e4m3) exactly (no double rounding).
    """
    u = np.ascontiguousarray(a, dtype=np.float32).view(np.uint32)
    hi = (u >> 16).astype(np.uint16)
    hi |= ((u & 0xFFFF) != 0).astype(np.uint16)
    return hi.view(ml_dtypes.bfloat16)


_NC = None


def _get_nc():
    global _NC
    if _NC is None:
        _NC = build_nc()
    return _NC


def kernel(input, weight, input_scale_e4m3=None, weight_scale_e4m3=None,
           **_unused):
    from concourse.bass_utils import run_bass_kernel_spmd

    x = np.asarray(input, dtype=np.float32).reshape(M_FULL, C_DIM)
    w = np.asarray(weight, dtype=np.float32)
    s_in = float(np.asarray(input_scale_e4m3)) if input_scale_e4m3 is not None else 1.0
    s_w = float(np.asarray(weight_scale_e4m3)) if weight_scale_e4m3 is not None else 1.0

    # reference semantics: round(x*s)/s etc.; fold scales on host (exact)
    if s_in != 1.0:
        x = x * s_in
    if s_w != 1.0:
        w = w * s_w

    # host-side staging: round-to-odd bf16 (exact w.r.t. the later fp8
    # RNE quantize) + transpose so c is the on-chip contraction dim
    xb = _round_to_odd_bf16(x)
    wb = _round_to_odd_bf16(w)
    xT = [np.ascontiguousarray(xb[mi * M_CORE:(mi + 1) * M_CORE].T)
          for mi in range(GRID_M)]
    wT = [np.ascontiguousarray(wb[oj * O_CORE:(oj + 1) * O_CORE].T)
          for oj in range(GRID_O)]

    nc = _get_nc()
    in_maps = []
    for mi in range(GRID_M):
        for oj in range(GRID_O):
            in_maps.append({"x_in": xT[mi], "w_in": wT[oj]})
    res = run_bass_kernel_spmd(nc, in_maps, core_ids=list(range(N_CORES)))

    out = np.empty((M_FULL, OUT_DIM), np.float32)
    for k, r in enumerate(res.results):
        mi, oj = divmod(k, GRID_O)
        out[mi * M_CORE:(mi + 1) * M_CORE, oj * O_CORE:(oj + 1) * O_CORE] = r["out"]

    inv = 1.0 / (s_in * s_w)
    if inv != 1.0:
        out = out * inv
    return out.reshape(A_DIM, B_DIM, OUT_DIM)
